# revision 25
# baseline (speedup 1.0000x reference)
"""BEVConvS Trainium2 kernel.

Rasterize 480k points into per-batch BEV grids (scatter-max via iterative
"peeling": plain indirect-DMA scatter rounds with gather-verify, which is
exact because every round's surviving writers exceed the round-start grid
value), then the conv stack as banded matmuls on the tensor engine.

Sharding: 8 cores, core c owns batch c//2 and grid rows
[496*(c%2), 496*(c%2)+528) of 1024. The 528-row slabs overlap by 32 rows so
every core rasterizes all rows its conv tower needs - no inter-core
communication. Host routes points by (batch, exact y-row), the device
recomputes the identical fp32 math and drops out-of-slab points.

Self-contained: only needs concourse (on PYTHONPATH in this container).
"""
import os
import sys
import time

for _p in ("/opt/trn_rl_repo", "/root/.axon_site/_ro/trn_rl_repo"):
    if os.path.isdir(_p) and _p not in sys.path:
        sys.path.append(_p)

import hashlib

import numpy as np

import concourse.bass as bass
import concourse.mybir as mybir
import concourse.tile as tile
from concourse.bass_utils import run_bass_kernel_spmd

F32 = mybir.dt.float32
I32 = mybir.dt.int32
AF = mybir.ActivationFunctionType
OP = mybir.AluOpType

# ---------------------------------------------------------------- geometry
B, N, H, W = 4, 480000, 1024, 1024
PR = (0.0, -39.68, -3.0, 69.12, 39.68, 1.0)
BN_EPS = 1e-5
X_SCALE = np.float32(W / (PR[3] - PR[0]))
Y_OFF = np.float32((PR[4] - PR[1]) / 2)
Y_SCALE = np.float32(H / (PR[4] - PR[1]))

R0_ODD = 496          # odd cores' first grid row (multiple of 8)
RROWS = 528           # grid rows per core
NPTS = 64000          # padded points per core
PCOLS = NPTS // 128   # 500

# grid: per-channel band-major planes [16 bands][530 rows][66 cols]
NB, GY, GX = 16, RROWS + 2, 66
PLANE = NB * GY * GX            # 559680
GSIZE = 2 * PLANE + 128
DUMP = 2 * PLANE                # dump cell for dropped points

NSURV = 8320                    # compacted survivor slots (128*65)
SCOLS = NSURV // 128            # 65
DUMPSLOT = NSURV                # dump slot OUTSIDE the reloaded region
PEEL_ROUNDS = 3                 # gather+rescatter rounds after round 2

TWO23 = float(2.0 ** 23)

_CACHE = {}


# ================================================================ device
def _floor(nc, pool, x, tag):
    """Exact floor via round-to-nearest (+2^23 trick) then correction."""
    r = pool.tile([128, x.shape[1]], F32, tag=tag + "_r")
    c = pool.tile([128, x.shape[1]], F32, tag=tag + "_c")
    nc.vector.tensor_scalar_add(r[:], x, TWO23)
    nc.vector.tensor_scalar_add(r[:], r[:], -TWO23)
    nc.vector.tensor_tensor(c[:], r[:], x, op=OP.is_gt)
    nc.vector.tensor_tensor(r[:], r[:], c[:], op=OP.subtract)
    return r


def _cmp_s(nc, pool, x, const, op, tag):
    m = pool.tile([128, x.shape[1]], F32, tag=tag)
    nc.vector.tensor_scalar(out=m[:], in0=x, scalar1=float(const), scalar2=None,
                            op0=op)
    return m



def _mk_ap(tensor, offset, dims):
    """Hand-built DRAM access pattern: dims = [[stride, count], ...]."""
    a = tensor.ap()
    a2 = a.copy()
    a2.ap = type(a.ap)(dims)
    a2.offset = int(offset)
    return a2


def build_nc():
    nc = bass.Bass()

    pts = nc.dram_tensor("pts", [128, 4 * PCOLS], F32, kind="ExternalInput")
    meta = nc.dram_tensor("meta", [128, 2], F32, kind="ExternalInput")
    wb1 = nc.dram_tensor("wb1", [96, 3 * 128], F32, kind="ExternalInput")
    wb2 = nc.dram_tensor("wb2", [96, 3 * 64], F32, kind="ExternalInput")
    wbd = nc.dram_tensor("wbd", [96, 3 * 64], F32, kind="ExternalInput")
    wb3 = nc.dram_tensor("wb3", [96, 3 * 64], F32, kind="ExternalInput")
    biases = nc.dram_tensor("biases", [128, 4], F32, kind="ExternalInput")
    ltri = nc.dram_tensor("ltri", [128, 128], F32, kind="ExternalInput")

    G = nc.dram_tensor("G", [GSIZE, 1], F32)
    surv = nc.dram_tensor("surv", [NSURV + 32, 4], F32)
    A1f = nc.dram_tensor("A1f", [32 * 266 * 130, 1], F32)       # 1106560
    A2f = nc.dram_tensor("A2f", [32 * 134 * 130, 1], F32)       # 557440
    A3f = nc.dram_tensor("A3f", [32 * 134 * 258, 1], F32)       # 1106304
    OUT = nc.dram_tensor("out", [64, 66, 128], F32, kind="ExternalOutput")
    DBG = os.environ.get("BEV_DEBUG") == "1"
    if DBG:
        dbgv = nc.dram_tensor("dbgv", [128, 3 * PCOLS], F32,
                              kind="ExternalOutput")
        dbgs = nc.dram_tensor("dbgs", [128, 4 * SCOLS], F32,
                              kind="ExternalOutput")
        dbgg = nc.dram_tensor("dbgg", [128, GSIZE // 128], F32,
                              kind="ExternalOutput")

    # structured views
    Gcb = G[0:2 * PLANE, 0:1].rearrange(
        "(cb y x) o -> cb y (x o)", cb=2 * NB, y=GY)          # [32, 530, 66]
    A1 = A1f[:, 0:1].rearrange("(r y x) o -> r y (x o)", r=32, y=266)
    A2 = A2f[:, 0:1].rearrange("(r y x) o -> r y (x o)", r=32, y=134)
    A3 = A3f[0:32 * 134 * 258, 0:1].rearrange(
        "(r y x) o -> r y (x o)", r=32, y=134)

    with tile.TileContext(nc) as tc:
        with (tc.tile_pool(name="sbuf", bufs=1) as pool,
              tc.tile_pool(name="io", bufs=3) as iop,
              tc.tile_pool(name="psum", bufs=2, space="PSUM") as pp):
            # ---------------- inputs
            pts_t = pool.tile([128, 4 * PCOLS], F32)
            meta_t = pool.tile([128, 2], F32)
            w1_t = pool.tile([96, 3 * 128], F32)
            w2_t = pool.tile([96, 3 * 64], F32)
            wd_t = pool.tile([96, 3 * 64], F32)
            w3_t = pool.tile([96, 3 * 64], F32)
            bias_t = pool.tile([128, 4], F32)
            ltri_t = pool.tile([128, 128], F32)
            nc.sync.dma_start(out=pts_t[:], in_=pts[:])
            nc.sync.dma_start(out=meta_t[:], in_=meta[:])
            nc.sync.dma_start(out=w1_t[:], in_=wb1[:])
            nc.sync.dma_start(out=w2_t[:], in_=wb2[:])
            nc.sync.dma_start(out=wd_t[:], in_=wbd[:])
            nc.sync.dma_start(out=w3_t[:], in_=wb3[:])
            nc.sync.dma_start(out=bias_t[:], in_=biases[:])
            nc.sync.dma_start(out=ltri_t[:], in_=ltri[:])

            px = pts_t[:, 0 * PCOLS:1 * PCOLS]
            py = pts_t[:, 1 * PCOLS:2 * PCOLS]
            pz = pts_t[:, 2 * PCOLS:3 * PCOLS]
            pi = pts_t[:, 3 * PCOLS:4 * PCOLS]

            # ---------------- init DRAM scratch
            ZC = 5000
            zt = pool.tile([128, ZC], F32)
            nc.vector.memset(zt[:], 0.0)

            def zero_dram(flat, n_elems):
                v = flat[0:n_elems, 0:1].rearrange(
                    "(p f) o -> p (f o)", p=128)
                cols = n_elems // 128
                for c0 in range(0, cols, ZC):
                    c1 = min(c0 + ZC, cols)
                    nc.sync.dma_start(out=v[:, c0:c1], in_=zt[:, :c1 - c0])

            zero_dram(G, GSIZE)
            zero_dram(A1f, 32 * 266 * 130)
            zero_dram(A2f, 32 * 134 * 130)
            zero_dram(A3f, 32 * 134 * 258)
            nc.sync.dma_start(
                out=surv[0:NSURV, :].rearrange("(p j) f -> p (j f)", p=128),
                in_=zt[:, :NSURV * 4 // 128])
            # z plane in-image cells = -10 (rows 1..528, cols 1..64 per band)
            neg10 = pool.tile([128, 2112], F32)
            nc.vector.memset(neg10[:], -10.0)
            for bb in range(0, NB, 8):
                nc.sync.dma_start(out=Gcb[bb:bb + 8, 1:529, 1:65],
                                  in_=neg10[:])

            # ---------------- per-point math
            xf = pool.tile([128, PCOLS], F32)
            yf = pool.tile([128, PCOLS], F32)
            nc.vector.tensor_scalar_mul(xf[:], px, float(X_SCALE))
            nc.vector.tensor_scalar_add(yf[:], py, float(Y_OFF))
            nc.vector.tensor_scalar_mul(yf[:], yf[:], float(Y_SCALE))
            xfl = _floor(nc, pool, xf[:], "xfl")
            yfl = _floor(nc, pool, yf[:], "yfl")

            g = pool.tile([128, PCOLS], F32)
            nc.vector.tensor_tensor(
                g[:], yfl[:], meta_t[:, 0:1].to_broadcast([128, PCOLS]),
                op=OP.subtract)

            bq = pool.tile([128, PCOLS], F32)
            nc.vector.tensor_scalar_mul(bq[:], xfl[:], 1.0 / 64.0)
            bfl = _floor(nc, pool, bq[:], "bfl")
            uu = pool.tile([128, PCOLS], F32)
            nc.vector.tensor_scalar_mul(uu[:], bfl[:], -64.0)
            nc.vector.tensor_tensor(uu[:], uu[:], xfl[:], op=OP.add)

            m = _cmp_s(nc, pool, xfl[:], 0.0, OP.is_ge, "macc")
            for const, op, tg, srcv in ((1023.0, OP.is_le, "cmps", xfl),
                                        (0.0, OP.is_ge, "cmps", yfl),
                                        (1023.0, OP.is_le, "cmps", yfl),
                                        (0.0, OP.is_ge, "cmps", g),
                                        (float(RROWS - 1), OP.is_le, "cmps", g)):
                mm = _cmp_s(nc, pool, srcv[:], const, op, tg)
                nc.vector.tensor_tensor(m[:], m[:], mm[:], op=OP.mult)

            # cell = b*GY*GX + g*GX + u + (GX+1)
            cellz = pool.tile([128, PCOLS], F32)
            tmpc = pool.tile([128, PCOLS], F32)
            nc.vector.tensor_scalar_mul(cellz[:], bfl[:], float(GY * GX))
            nc.vector.tensor_scalar_mul(tmpc[:], g[:], float(GX))
            nc.vector.tensor_tensor(cellz[:], cellz[:], tmpc[:], op=OP.add)
            nc.vector.tensor_tensor(cellz[:], cellz[:], uu[:], op=OP.add)
            nc.vector.tensor_scalar_add(cellz[:], cellz[:], float(GX + 1))
            dump_t = pool.tile([128, 1], F32)
            nc.vector.memset(dump_t[:], float(DUMP))
            m8 = pool.tile([128, PCOLS], mybir.dt.uint8)
            nc.vector.tensor_copy(m8[:], m[:])
            celli = pool.tile([128, PCOLS], F32)
            nc.vector.tensor_scalar_add(celli[:], cellz[:], float(PLANE))
            cellzs = pool.tile([128, PCOLS], F32)
            cellis = pool.tile([128, PCOLS], F32)
            nc.vector.select(cellzs[:], m8[:], cellz[:],
                             dump_t[:, 0:1].to_broadcast([128, PCOLS]))
            nc.vector.select(cellis[:], m8[:], celli[:],
                             dump_t[:, 0:1].to_broadcast([128, PCOLS]))
            cellz, celli = cellzs, cellis

            iz = pool.tile([128, PCOLS], I32)
            ii = pool.tile([128, PCOLS], I32)
            nc.vector.tensor_copy(iz[:], cellz[:])
            nc.vector.tensor_copy(ii[:], celli[:])

            def scat(idx, val, j):
                nc.gpsimd.indirect_dma_start(
                    out=G[:],
                    out_offset=bass.IndirectOffsetOnAxis(ap=idx[:, j:j + 1], axis=0),
                    in_=val[:, j:j + 1], in_offset=None)

            def gath(idx, dst, j):
                nc.gpsimd.indirect_dma_start(
                    out=dst[:, j:j + 1], out_offset=None, in_=G[:],
                    in_offset=bass.IndirectOffsetOnAxis(ap=idx[:, j:j + 1], axis=0))

            # ---------------- round 1: scatter all, gather-verify
            for j in range(PCOLS):
                scat(iz, pz, j)
            for j in range(PCOLS):
                scat(ii, pi, j)
            gz = pool.tile([128, PCOLS], F32)
            gi = pool.tile([128, PCOLS], F32)
            for j in range(PCOLS):
                gath(iz, gz, j)
            for j in range(PCOLS):
                gath(ii, gi, j)

            uz = pool.tile([128, PCOLS], F32)
            ui = pool.tile([128, PCOLS], F32)
            nc.vector.tensor_tensor(uz[:], pz, gz[:], op=OP.is_gt)
            nc.vector.tensor_tensor(ui[:], pi, gi[:], op=OP.is_gt)
            um = pool.tile([128, PCOLS], F32)
            nc.vector.tensor_tensor(um[:], uz[:], ui[:], op=OP.max)
            if DBG:
                nc.sync.dma_start(out=dbgv.ap()[:, 0:PCOLS], in_=gz[:])
                nc.sync.dma_start(out=dbgv.ap()[:, PCOLS:2 * PCOLS], in_=gi[:])
                nc.sync.dma_start(out=dbgv.ap()[:, 2 * PCOLS:3 * PCOLS],
                                  in_=um[:])

            # ---------------- compact survivors (cell, z, i)
            ca = pool.tile([128, PCOLS], F32)
            cb = pool.tile([128, PCOLS], F32)
            nc.vector.tensor_copy(ca[:], um[:])
            cur, nxt = ca, cb
            sh = 1
            while sh < PCOLS:
                nc.vector.tensor_copy(nxt[:, :sh], cur[:, :sh])
                nc.vector.tensor_tensor(nxt[:, sh:], cur[:, sh:],
                                        cur[:, :PCOLS - sh], op=OP.add)
                cur, nxt = nxt, cur
                sh *= 2
            excl = pool.tile([128, PCOLS], F32)
            nc.vector.tensor_tensor(excl[:], cur[:], um[:], op=OP.subtract)
            cnt = pool.tile([128, 1], F32)
            nc.vector.tensor_reduce(cnt[:], um[:], op=OP.add,
                                    axis=mybir.AxisListType.X)
            pref_ps = pp.tile([128, 1], F32, space="PSUM", tag="mm1")
            nc.tensor.matmul(pref_ps[:], lhsT=ltri_t[:], rhs=cnt[:],
                             start=True, stop=True)
            pref = pool.tile([128, 1], F32)
            nc.vector.tensor_copy(pref[:], pref_ps[:])
            slot = pool.tile([128, PCOLS], F32)
            nc.vector.tensor_tensor(
                slot[:], excl[:], pref[:, 0:1].to_broadcast([128, PCOLS]),
                op=OP.add)
            dslot = pool.tile([128, 1], F32)
            nc.vector.memset(dslot[:], float(DUMPSLOT))
            um8 = pool.tile([128, PCOLS], mybir.dt.uint8)
            nc.vector.tensor_copy(um8[:], um[:])
            slots = pool.tile([128, PCOLS], F32)
            nc.vector.select(slots[:], um8[:], slot[:],
                             dslot[:, 0:1].to_broadcast([128, PCOLS]))
            nc.vector.tensor_scalar(out=slots[:], in0=slots[:],
                                    scalar1=float(DUMPSLOT), scalar2=None,
                                    op0=OP.min)
            slotI = pool.tile([128, PCOLS], I32)
            nc.vector.tensor_copy(slotI[:], slots[:])

            uz8 = pool.tile([128, PCOLS], mybir.dt.uint8)
            ui8 = pool.tile([128, PCOLS], mybir.dt.uint8)
            nc.vector.tensor_copy(uz8[:], uz[:])
            nc.vector.tensor_copy(ui8[:], ui[:])
            zc = pool.tile([128, PCOLS], F32)
            ic = pool.tile([128, PCOLS], F32)
            nc.vector.select(zc[:], uz8[:], cellz[:],
                             dump_t[:, 0:1].to_broadcast([128, PCOLS]))
            nc.vector.select(ic[:], ui8[:], celli[:],
                             dump_t[:, 0:1].to_broadcast([128, PCOLS]))
            q = pool.tile([128, 4 * PCOLS], F32)
            qv = q[:].rearrange("p (j four) -> p j four", four=4)
            nc.vector.tensor_copy(qv[:, :, 0], zc[:])
            nc.vector.tensor_copy(qv[:, :, 1], pz)
            nc.vector.tensor_copy(qv[:, :, 2], ic[:])
            nc.vector.tensor_copy(qv[:, :, 3], pi)
            for j in range(PCOLS):
                nc.gpsimd.indirect_dma_start(
                    out=surv[:],
                    out_offset=bass.IndirectOffsetOnAxis(
                        ap=slotI[:, j:j + 1], axis=0),
                    in_=q[:, 4 * j:4 * j + 4], in_offset=None)

            # ---------------- rounds 2+: rescatter survivors
            s4 = pool.tile([128, SCOLS * 4], F32)
            nc.sync.dma_start(
                out=s4[:],
                in_=surv[0:NSURV, :].rearrange("(p j) f -> p (j f)", p=128))
            s4v = s4[:].rearrange("p (j four) -> p j four", four=4)
            scf = pool.tile([128, SCOLS], F32)
            sz = pool.tile([128, SCOLS], F32)
            sif = pool.tile([128, SCOLS], F32)
            si = pool.tile([128, SCOLS], F32)
            nc.vector.tensor_copy(scf[:], s4v[:, :, 0])
            nc.vector.tensor_copy(sz[:], s4v[:, :, 1])
            nc.vector.tensor_copy(sif[:], s4v[:, :, 2])
            nc.vector.tensor_copy(si[:], s4v[:, :, 3])

            sidz = pool.tile([128, SCOLS], I32)
            sidi = pool.tile([128, SCOLS], I32)
            nc.vector.tensor_copy(sidz[:], scf[:])
            nc.vector.tensor_copy(sidi[:], sif[:])
            if DBG:
                nc.sync.dma_start(out=dbgs.ap()[:, :], in_=s4[:])

            gz2 = pool.tile([128, SCOLS], F32)
            gi2 = pool.tile([128, SCOLS], F32)
            uz2 = pool.tile([128, SCOLS], F32)
            ui2 = pool.tile([128, SCOLS], F32)
            mzc = pool.tile([128, SCOLS], F32)
            mic = pool.tile([128, SCOLS], F32)
            uz28 = pool.tile([128, SCOLS], mybir.dt.uint8)
            ui28 = pool.tile([128, SCOLS], mybir.dt.uint8)

            for j in range(SCOLS):
                scat(sidz, sz, j)
            for j in range(SCOLS):
                scat(sidi, si, j)
            for _r in range(PEEL_ROUNDS):
                for j in range(SCOLS):
                    gath(sidz, gz2, j)
                for j in range(SCOLS):
                    gath(sidi, gi2, j)
                nc.vector.tensor_tensor(uz2[:], sz[:], gz2[:], op=OP.is_gt)
                nc.vector.tensor_tensor(ui2[:], si[:], gi2[:], op=OP.is_gt)
                nc.vector.tensor_copy(uz28[:], uz2[:])
                nc.vector.tensor_copy(ui28[:], ui2[:])
                nc.vector.select(mzc[:], uz28[:], scf[:],
                                 dump_t[:, 0:1].to_broadcast([128, SCOLS]))
                nc.vector.select(mic[:], ui28[:], sif[:],
                                 dump_t[:, 0:1].to_broadcast([128, SCOLS]))
                nc.vector.tensor_copy(sidz[:], mzc[:])
                nc.vector.tensor_copy(sidi[:], mic[:])
                for j in range(SCOLS):
                    scat(sidz, sz, j)
                for j in range(SCOLS):
                    scat(sidi, si, j)

            # ---------------- band halo columns
            with nc.allow_non_contiguous_dma(reason="halo cols"):
                for c in range(2):
                    nc.sync.dma_start(
                        out=Gcb[16 * c + 1:16 * c + 16, :, 0:1],
                        in_=Gcb[16 * c:16 * c + 15, :, 64:65])
                    nc.sync.dma_start(
                        out=Gcb[16 * c:16 * c + 15, :, 65:66],
                        in_=Gcb[16 * c + 1:16 * c + 16, :, 1:2])

            if DBG:
                nc.sync.dma_start(
                    out=dbgg.ap()[:, :],
                    in_=G.ap().rearrange("(p f) o -> p (f o)", p=128))

            # ================================================ conv tower
            # conv1: 2ch -> 8ch on [530,1024]; K=(dy,ci2,b16)=96,
            # M=(co8,b16)=128, then 2x2 pool -> A1[(co,bhi4)=32,266,130]
            bias1 = bias_t[:, 0:1]
            for ch in range(66):
                y0 = 8 * ch
                rhs = iop.tile([96, 8 * 66], F32, tag="rhs1")
                for dy in range(3):
                    nc.sync.dma_start(
                        out=rhs[32 * dy:32 * dy + 32, :],
                        in_=Gcb[:, y0 + dy:y0 + dy + 8, :])
                ps = pp.tile([128, 512], F32, space="PSUM", tag="mm1")
                rv = rhs[:].rearrange("k (y x) -> k y x", y=8)
                for dx in range(3):
                    nc.tensor.matmul(
                        ps[:], lhsT=w1_t[:, 128 * dx:128 * dx + 128],
                        rhs=rv[:, :, dx:dx + 64],
                        start=(dx == 0), stop=(dx == 2))
                act = iop.tile([128, 512], F32, tag="act1")
                nc.scalar.activation(act[:], ps[:], AF.Relu, bias=bias1,
                                     scale=1.0)
                av = act[:].rearrange("m (y u) -> m y u", y=8)
                p1 = iop.tile([128, 4 * 64], F32, tag="p1a")
                p1v = p1[:].rearrange("m (y u) -> m y u", y=4)
                nc.vector.tensor_tensor(p1v[:, :, :], av[:, 0:8:2, :],
                                        av[:, 1:8:2, :], op=OP.max)
                p2 = iop.tile([128, 4 * 32], F32, tag="p1b")
                p2v = p2[:].rearrange("m (y u) -> m y u", y=4)
                nc.vector.tensor_tensor(p2v[:, :, :], p1v[:, :, 0:64:2],
                                        p1v[:, :, 1:64:2], op=OP.max)
                for yy in range(4):
                    nc.sync.dma_start(
                        out=_mk_ap(A1f, (1 + 4 * ch + yy) * 130 + 1,
                                   [[34580, 32], [32, 4], [1, 32]]),
                        in_=p2v[:, yy, :])

            # A1 inter-band halo columns
            A1q = A1f[:, 0:1].rearrange(
                "(co bhi y x) o -> co bhi y (x o)", co=8, bhi=4, y=266)
            with nc.allow_non_contiguous_dma(reason="halo cols"):
                nc.sync.dma_start(out=A1q[:, 1:4, :, 0:1],
                                  in_=A1q[:, 0:3, :, 128:129])
                nc.sync.dma_start(out=A1q[:, 0:3, :, 129:130],
                                  in_=A1q[:, 1:4, :, 1:2])

            # conv2: 8->16 grouped(8) on [264,512]; K=(dy,ci8,bhi4)=96,
            # M=(co16,b4)=64, pool -> A2[(co,bhi2)=32,134,130]
            bias2 = bias_t[0:64, 1:2]
            for ch in range(66):
                y0 = 4 * ch
                rhs = iop.tile([96, 4 * 130], F32, tag="rhs2")
                for dy in range(3):
                    nc.sync.dma_start(
                        out=rhs[32 * dy:32 * dy + 32, :],
                        in_=A1[:, y0 + dy:y0 + dy + 4, :])
                ps = pp.tile([64, 512], F32, space="PSUM", tag="mm1")
                rv = rhs[:].rearrange("k (y x) -> k y x", y=4)
                for dx in range(3):
                    nc.tensor.matmul(
                        ps[:], lhsT=w2_t[:, 64 * dx:64 * dx + 64],
                        rhs=rv[:, :, dx:dx + 128],
                        start=(dx == 0), stop=(dx == 2))
                act = iop.tile([64, 512], F32, tag="act2")
                nc.scalar.activation(act[:], ps[:], AF.Relu, bias=bias2,
                                     scale=1.0)
                av = act[:].rearrange("m (y u) -> m y u", y=4)
                p1 = iop.tile([64, 2 * 128], F32, tag="p2a")
                p1v = p1[:].rearrange("m (y u) -> m y u", y=2)
                nc.vector.tensor_tensor(p1v[:, :, :], av[:, 0:4:2, :],
                                        av[:, 1:4:2, :], op=OP.max)
                p2 = iop.tile([64, 2 * 64], F32, tag="p2b")
                p2v = p2[:].rearrange("m (y u) -> m y u", y=2)
                nc.vector.tensor_tensor(p2v[:, :, :], p1v[:, :, 0:128:2],
                                        p1v[:, :, 1:128:2], op=OP.max)
                for yy in range(2):
                    nc.sync.dma_start(
                        out=_mk_ap(A2f, (1 + 2 * ch + yy) * 130 + 1,
                                   [[17420, 32], [64, 2], [1, 64]]),
                        in_=p2v[:, yy, :])

            # A2 inter-band halo columns
            A2q = A2f[:, 0:1].rearrange(
                "(co bhi y x) o -> co bhi y (x o)", co=16, bhi=2, y=134)
            with nc.allow_non_contiguous_dma(reason="halo cols"):
                nc.sync.dma_start(out=A2q[:, 1:2, :, 0:1],
                                  in_=A2q[:, 0:1, :, 128:129])
                nc.sync.dma_start(out=A2q[:, 0:1, :, 129:130],
                                  in_=A2q[:, 1:2, :, 1:2])

            # dw+pw fused: 16->32 dense 3x3 on [132,256]; K=(dy,ci16,bhi2)=96,
            # M=(co32,b2)=64 -> A3[co32,134,258]
            biasd = bias_t[0:64, 2:3]
            for ch in range(33):
                y0 = 4 * ch
                rhs = iop.tile([96, 4 * 130], F32, tag="rhsd")
                for dy in range(3):
                    nc.sync.dma_start(
                        out=rhs[32 * dy:32 * dy + 32, :],
                        in_=A2[:, y0 + dy:y0 + dy + 4, :])
                ps = pp.tile([64, 512], F32, space="PSUM", tag="mm1")
                rv = rhs[:].rearrange("k (y x) -> k y x", y=4)
                for dx in range(3):
                    nc.tensor.matmul(
                        ps[:], lhsT=wd_t[:, 64 * dx:64 * dx + 64],
                        rhs=rv[:, :, dx:dx + 128],
                        start=(dx == 0), stop=(dx == 2))
                act = iop.tile([64, 512], F32, tag="actd")
                nc.scalar.activation(act[:], ps[:], AF.Relu, bias=biasd,
                                     scale=1.0)
                av = act[:].rearrange("m (y u) -> m y u", y=4)
                for yy in range(4):
                    nc.sync.dma_start(
                        out=_mk_ap(A3f, (1 + y0 + yy) * 258 + 1,
                                   [[34572, 32], [128, 2], [1, 128]]),
                        in_=av[:, yy, :])

            # conv3: 32->64 on [132,256]; K=(dy,ci32)=96, M=co64,
            # pool -> OUT[64,66,128]
            bias3 = bias_t[0:64, 3:4]
            for ch in range(66):
                y0 = 2 * ch
                rhs = iop.tile([96, 2 * 258], F32, tag="rhs3")
                for dy in range(3):
                    nc.sync.dma_start(
                        out=rhs[32 * dy:32 * dy + 32, :],
                        in_=A3[:, y0 + dy:y0 + dy + 2, :])
                ps = pp.tile([64, 512], F32, space="PSUM", tag="mm1")
                rv = rhs[:].rearrange("k (y x) -> k y x", y=2)
                for dx in range(3):
                    nc.tensor.matmul(
                        ps[:], lhsT=w3_t[:, 64 * dx:64 * dx + 64],
                        rhs=rv[:, :, dx:dx + 256],
                        start=(dx == 0), stop=(dx == 2))
                act = iop.tile([64, 512], F32, tag="act3")
                nc.scalar.activation(act[:], ps[:], AF.Relu, bias=bias3,
                                     scale=1.0)
                av = act[:].rearrange("m (y u) -> m y u", y=2)
                p1 = iop.tile([64, 256], F32, tag="p3a")
                nc.vector.tensor_tensor(p1[:], av[:, 0, :], av[:, 1, :],
                                        op=OP.max)
                p2 = iop.tile([64, 128], F32, tag="p3b")
                nc.vector.tensor_tensor(p2[:], p1[:, 0:256:2], p1[:, 1:256:2],
                                        op=OP.max)
                nc.sync.dma_start(out=OUT.ap()[:, ch, :], in_=p2[:])

    _legalize_waits(nc)
    return nc


# ============================================================ wait fixup
def _legalize_waits(nc, max_waits=1):
    """This walrus build encodes a single sync-wait slot per ISA
    instruction; hoist extra waits onto same-engine NoOps just before."""
    n = 0
    for fn in nc.m.functions:
        for blk in fn.blocks:
            new_insts = []
            for inst in blk.instructions:
                si = inst.sync_info
                waits = list(si.on_wait) if (si is not None and si.on_wait) else []
                if len(waits) > max_waits:
                    for w in waits[:-max_waits]:
                        n += 1
                        new_insts.append(mybir.InstNoOp(
                            name=f"WSPLIT-{n}",
                            engine=inst.engine,
                            text_hint="wait_split",
                            sync_info=mybir.SyncInfo(on_wait=[w], on_update=[])))
                    si.on_wait = waits[-max_waits:]
                new_insts.append(inst)
            blk.instructions = new_insts
    return n


# ============================================================ host side
def _bn_fold(g, be, m, v):
    s = (np.asarray(g, np.float32) /
         np.sqrt(np.asarray(v, np.float32) + np.float32(BN_EPS)))
    t = np.asarray(be, np.float32) - np.asarray(m, np.float32) * s
    return s.astype(np.float32), t.astype(np.float32)


def _prep_weights(inp):
    f = lambda k: np.asarray(inp[k], np.float32)
    s0, t0 = _bn_fold(f('g0'), f('be0'), f('m0'), f('v0'))
    s1, t1 = _bn_fold(f('g1'), f('be1'), f('m1'), f('v1'))
    s2, t2 = _bn_fold(f('g2'), f('be2'), f('m2'), f('v2'))
    s3, t3 = _bn_fold(f('g3'), f('be3'), f('m3'), f('v3'))

    w0 = f('w0') * s0[:, None, None, None]          # [8,2,3,3]
    b0 = f('b0') * s0 + t0
    w1 = f('w1') * s1[:, None, None, None]          # [16,1,3,3]
    b1 = f('b1') * s1 + t1
    # dw (16 dw) then pw 16->32, then BN2: combined 3x3 dense 16->32
    wdw, bdw, wpw, bpw = f('wdw'), f('bdw'), f('wpw'), f('bpw')
    wdp = np.einsum('oi,ikl->oikl', wpw[:, :, 0, 0], wdw[:, 0])  # [32,16,3,3]
    bdp = bpw + wpw[:, :, 0, 0] @ bdw
    wdp = wdp * s2[:, None, None, None]
    bdp = bdp * s2 + t2
    w3 = f('w3') * s3[:, None, None, None]          # [64,32,3,3]
    b3 = f('b3') * s3 + t3

    # conv1 lhsT: k=(dy,ci2,b16), m=(co8,b16)
    wb1 = np.zeros((96, 3, 128), np.float32)
    for dy in range(3):
        for ci in range(2):
            for b in range(16):
                k = dy * 32 + ci * 16 + b
                for co in range(8):
                    for dx in range(3):
                        wb1[k, dx, co * 16 + b] = w0[co, ci, dy, dx]
    # conv2 lhsT: k=(dy,ci8,bhi4), m=(co16,b4); grouped: ci==co//2
    wb2 = np.zeros((96, 3, 64), np.float32)
    for dy in range(3):
        for ci in range(8):
            for b in range(4):
                k = dy * 32 + ci * 4 + b
                for co in range(16):
                    if co // 2 != ci:
                        continue
                    for dx in range(3):
                        wb2[k, dx, co * 4 + b] = w1[co, 0, dy, dx]
    # dwpw lhsT: k=(dy,ci16,bhi2), m=(co32,b2)
    wbd = np.zeros((96, 3, 64), np.float32)
    for dy in range(3):
        for ci in range(16):
            for b in range(2):
                k = dy * 32 + ci * 2 + b
                for co in range(32):
                    for dx in range(3):
                        wbd[k, dx, co * 2 + b] = wdp[co, ci, dy, dx]
    # conv3 lhsT: k=(dy,ci32), m=co64
    wb3 = np.zeros((96, 3, 64), np.float32)
    for dy in range(3):
        for ci in range(32):
            k = dy * 32 + ci
            for co in range(64):
                for dx in range(3):
                    wb3[k, dx, co] = w3[co, ci, dy, dx]

    biases = np.zeros((128, 4), np.float32)
    biases[:, 0] = np.repeat(b0, 16)
    biases[:64, 1] = np.repeat(b1, 4)
    biases[:64, 2] = np.repeat(bdp, 2)
    biases[:64, 3] = b3

    ltri = np.triu(np.ones((128, 128), np.float32), 1)
    return {
        "wb1": wb1.reshape(96, 384).copy(),
        "wb2": wb2.reshape(96, 192).copy(),
        "wbd": wbd.reshape(96, 192).copy(),
        "wb3": wb3.reshape(96, 192).copy(),
        "biases": biases, "ltri": ltri,
    }


def _route_points(points):
    """Split points into 8 per-core packed arrays."""
    pts = np.asarray(points, np.float32)
    bidx = pts[:, 0].astype(np.int32)
    px, py = pts[:, 1], pts[:, 2]
    xfl = np.floor(px * X_SCALE)
    yfl = np.floor((py + Y_OFF) * Y_SCALE)
    valid = (xfl >= 0) & (xfl <= 1023) & (yfl >= 0) & (yfl <= 1023)
    packs = []
    for c in range(8):
        b, odd = c // 2, c % 2
        r0 = R0_ODD * odd
        sel = valid & (bidx == b) & (yfl >= r0) & (yfl < r0 + RROWS)
        n = int(sel.sum())
        assert n <= NPTS, f"core {c}: {n} points > {NPTS}"
        arr = np.empty((4, NPTS), np.float32)
        arr[0, :n] = px[sel]
        arr[1, :n] = py[sel]
        arr[2, :n] = pts[sel, 3]
        arr[3, :n] = pts[sel, 4]
        arr[0, n:] = -1.0e9
        arr[1, n:] = 0.0
        arr[2, n:] = -1.0e30
        arr[3, n:] = -1.0e30
        packed = arr.reshape(4, 128, PCOLS).transpose(1, 0, 2).reshape(
            128, 4 * PCOLS).copy()
        meta = np.zeros((128, 2), np.float32)
        meta[:, 0] = float(r0)
        packs.append((packed, meta))
    return packs


LAST_HW_EXEC_NS = None


def _get_nc():
    if "nc" in _CACHE:
        return _CACHE["nc"]
    try:
        with open(__file__, "rb") as f:
            ver = hashlib.sha256(f.read()).hexdigest()[:16]
    except OSError:
        ver = "nover"
    path = f"/tmp/bev_bir_{ver}.json"
    nc = None
    if os.path.exists(path):
        try:
            with open(path, "rb") as f:
                bj = f.read()
            nc = bass.Bass()
            nc.m = mybir.module_from_json_bytes(bj)
        except Exception:
            nc = None
    if nc is None:
        nc = build_nc()
        try:
            tmp = path + f".tmp{os.getpid()}"
            with open(tmp, "wb") as f:
                f.write(nc.to_json_bytes())
            os.replace(tmp, path)
        except OSError:
            pass
    _CACHE["nc"] = nc
    return nc


def kernel(points, batch_size, **kw):
    global LAST_HW_EXEC_NS
    nc = _get_nc()

    wmaps = _prep_weights(kw)
    packs = _route_points(points)
    in_maps = [dict(wmaps, pts=p, meta=m) for p, m in packs]

    t0 = time.time()
    res = run_bass_kernel_spmd(nc, in_maps, core_ids=list(range(8)))
    LAST_HW_EXEC_NS = int((time.time() - t0) * 1e9)

    out = np.empty((4, 64, 128, 128), np.float32)
    for c in range(8):
        b, odd = c // 2, c % 2
        o = res.results[c]["out"]            # [64, 66, 128]
        rows = o[:, 2:66, :] if odd else o[:, 0:64, :]
        out[b, :, 64 * odd:64 * odd + 64, :] = rows
    return out
